# revision 45
# baseline (speedup 1.0000x reference)
"""MultiHeadLatentAttention TRN2 kernel — 8-core batch(2) x head-group(4) parallel.

v2 strategy (rewrite of the 534us head-sharded baseline):
  - Sharding: core c handles batch c//4 and heads 4*(c%4)..4*(c%4)+3.
    Per-core x input halves (8.4MB) and the out-projection partial halves
    ([2048, 2048] instead of [4096, 2048]); host sums 4 partials per batch.
  - Host fuses latent down-projections into the up-projections (weight
    absorption) exactly like v1; 1/sqrt(HD) folded into q weights. All
    device operands pre-packed into their SBUF layouts so every DMA moves
    contiguous >=2KB rows; DMA issue order is arranged so the first
    projection matmul is gated on ~1.5MB, not the full weight set.
  - Projections: per 512-token sub-chunk, 12 accumulation chains
    (4 heads x q/k/v) at N=512. RoPE rotation via PE permutation matmul +
    DVE mul/add; the rotated shared rope-key is computed once (head 0) and
    copied to heads 1-3 instead of re-roped.
  - Attention: scores are computed TRANSPOSED (scoresT[k,q] = kT_blk^T @ qT)
    in quads of 4 q-blocks (N up to 512), exp'd on ACT straight into bf16
    SBUF, causal-masked multiplicatively (0/1 tri), and fed directly to the
    AV matmul (ctxT[hd,q] += vN_blk^T @ expT) — no attention transposes, no
    PSUM round-trips. Softmax denominators: either GpSimd
    partition_all_reduce over a DVE-accumulated exp-sum (MLA_RS=gp, default)
    or M=1 ones-matmuls on PE (MLA_RS=pe). Normalization is folded into the
    ctx PSUM->SBUF copy (tensor_tensor mult with the broadcast reciprocal).
  - Out-projection per quad, delayed by one quad so its PE burst overlaps
    the next quad's softmax latency; partials stream out as bf16
    (MLA_OUT=bf16 default; f32 fallback).
"""
import functools
import os
import numpy as np

import concourse.bass as bass
import concourse.mybir as mybir
from concourse.tile import TileContext
from concourse.bass_utils import run_bass_kernel_spmd

F32 = mybir.dt.float32
AF = mybir.ActivationFunctionType
ALU = mybir.AluOpType

NC = 8           # cores
NB = 2           # batch shards
NG = 4           # head-group shards
HL = 4           # heads per core
B, S, D = 2, 2048, 2048
H, HD, RD, CD = 16, 128, 64, 64
ND = D // 128    # 16 contraction chunks
SUB = 512        # projection sub-chunk (tokens, = PSUM N = xt stage)
NSUB = S // SUB  # 4
QB = 128
NQB = S // QB    # 16
NQUAD = NQB // 4  # 4 quads of 4 q-blocks

_DT_NAME = os.environ.get("MLA_DT", "bf16")     # bf16 | f32
_OUT_NAME = os.environ.get("MLA_OUT", "bf16")   # bf16 | f32

_CACHE = {}


# ---------------------------------------------------------------------------
# Tile-on-this-walrus compat: max ONE sync wait per instruction. Extra waits
# are hoisted onto wait-only EventSemaphore instructions inserted just before
# the over-subscribed instruction on the same engine (program order makes
# this equivalent).
# ---------------------------------------------------------------------------
def _split_multi_waits(nc, max_waits=1):
    n = 0
    for f in nc.m.functions:
        for bb in f.blocks:
            new_insts = []
            for ins in bb.instructions:
                si = ins.sync_info
                waits = list(si.on_wait) if si is not None else []
                if len(waits) > max_waits:
                    extra, keep = waits[:-max_waits], waits[-max_waits:]
                    for j, w in enumerate(extra):
                        ev = mybir.InstEventSemaphore(
                            name=f"{ins.name}_xw{j}",
                            engine=ins.engine,
                            ins=[],
                            outs=[],
                            sync_info=mybir.SyncInfo(on_wait=[w], on_update=[]),
                        )
                        new_insts.append(ev)
                        n += 1
                    ins.sync_info = mybir.SyncInfo(
                        on_wait=keep, on_update=list(si.on_update)
                    )
                new_insts.append(ins)
            bb.instructions[:] = new_insts
    return n


def _stream_dt():
    return mybir.dt.bfloat16 if _DT_NAME == "bf16" else mybir.dt.float32


def _out_dt():
    return mybir.dt.bfloat16 if _OUT_NAME == "bf16" else mybir.dt.float32


# ---------------------------------------------------------------------------
# Device program (SPMD: identical on all 8 cores, inputs differ per core)
# ---------------------------------------------------------------------------
def _build_program():
    DT = _stream_dt()
    ODT = _out_dt()
    nc = bass.Bass()

    xtp = nc.dram_tensor("xtp", [128, ND, S], DT, kind="ExternalInput")
    wqp = nc.dram_tensor("wqp", [128, ND, HL * HD], DT, kind="ExternalInput")
    wkp = nc.dram_tensor("wkp", [128, ND, HL * HD], DT, kind="ExternalInput")
    wvp = nc.dram_tensor("wvp", [128, ND, HL * HD], DT, kind="ExternalInput")
    wop = nc.dram_tensor("wop", [128, HL, D], DT, kind="ExternalInput")
    cosd = nc.dram_tensor("cosd", [RD, S], DT, kind="ExternalInput")
    sind = nc.dram_tensor("sind", [RD, S], DT, kind="ExternalInput")
    identd = nc.dram_tensor("identd", [128, 128], DT, kind="ExternalInput")
    p64d = nc.dram_tensor("p64d", [RD, RD], DT, kind="ExternalInput")
    trid = nc.dram_tensor("trid", [128, 128], DT, kind="ExternalInput")
    onesd = nc.dram_tensor("onesd", [128, 128], DT, kind="ExternalInput")
    outd = nc.dram_tensor("out", [S, D], ODT, kind="ExternalOutput")

    with TileContext(nc) as tc:
        with tc.tile_pool(name="stat", bufs=1) as stat, \
             tc.tile_pool(name="seq", bufs=1) as seq, \
             tc.tile_pool(name="xtc", bufs=2) as xtc, \
             tc.tile_pool(name="atp", bufs=9) as atp, \
             tc.tile_pool(name="ctxp", bufs=2) as ctxp, \
             tc.tile_pool(name="stream", bufs=4) as stream, \
             tc.tile_pool(name="small", bufs=2) as small, \
             tc.tile_pool(name="psA", bufs=2, space="PSUM") as psA, \
             tc.tile_pool(name="scr", bufs=3, space="PSUM") as scr, \
             tc.tile_pool(name="accp", bufs=2, space="PSUM") as accp:

            # ---- staged constants/weights; DMA order = priority order ----
            # (first projection chain is gated only on xt[d0:4] + wq[d0:4])
            def stage_xt(sub):
                xt = xtc.tile([128, ND, SUB], DT, tag="xt", name=f"xt{sub}")
                csl = slice(sub * SUB, (sub + 1) * SUB)
                for dpart in range(0, ND, 4):
                    dsl = slice(dpart, dpart + 4)
                    nc.sync.dma_start(out=xt[:, dsl, :], in_=xtp[:, dsl, csl])
                return xt

            wq_sb = stat.tile([128, ND, HL * HD], DT, tag="wq")
            wk_sb = stat.tile([128, ND, HL * HD], DT, tag="wk")
            wv_sb = stat.tile([128, ND, HL * HD], DT, tag="wv")
            xt0 = xtc.tile([128, ND, SUB], DT, tag="xt", name="xt0")
            for dpart in range(0, ND, 4):
                dsl = slice(dpart, dpart + 4)
                nc.sync.dma_start(out=xt0[:, dsl, :], in_=xtp[:, dsl, 0:SUB])
                nc.sync.dma_start(out=wq_sb[:, dsl, :], in_=wqp[:, dsl, :])
                nc.sync.dma_start(out=wk_sb[:, dsl, :], in_=wkp[:, dsl, :])
                nc.sync.dma_start(out=wv_sb[:, dsl, :], in_=wvp[:, dsl, :])
            cosT = stat.tile([128, S], DT, tag="cos")
            sinT = stat.tile([128, S], DT, tag="sin")
            nc.sync.dma_start(out=cosT[64:128, :], in_=cosd[:])
            nc.sync.dma_start(out=sinT[64:128, :], in_=sind[:])
            ident = stat.tile([128, 128], DT, tag="ident")
            nc.sync.dma_start(out=ident[:], in_=identd[:])
            p64 = stat.tile([128, RD], DT, tag="p64")
            nc.sync.dma_start(out=p64[64:128, :], in_=p64d[:])
            tri = stat.tile([128, 128], DT, tag="tri")
            nc.sync.dma_start(out=tri[:], in_=trid[:])
            ones = stat.tile([128, 128], DT, tag="ones")
            nc.sync.dma_start(out=ones[:], in_=onesd[:])
            wo_sb = stat.tile([128, HL, D], DT, tag="wo")

            qT = [seq.tile([128, S], DT, tag=f"qT{l}", name=f"qT{l}")
                  for l in range(HL)]
            kT = [seq.tile([128, S], DT, tag=f"kT{l}", name=f"kT{l}")
                  for l in range(HL)]
            vN = [seq.tile([128, S], DT, tag=f"vN{l}", name=f"vN{l}")
                  for l in range(HL)]

            # ---------------- projections ----------------
            # chain order per sub: q(0..3), k(0..3), v(0..3) — the first 4
            # chains are gated only on wq+xt (4MB); wk/wv stream in their
            # shadow. RoPE rotations ride along the k chains; v transposes
            # are deferred one chain so the vt drain never stalls PE.
            def chain(w_sb, l, xt):
                ps = psA.tile([128, SUB], F32, tag="mm", name="ps")
                for d in range(ND):
                    nc.tensor.matmul(ps[:], w_sb[:, d, l * 128:(l + 1) * 128],
                                     xt[:, d, :], start=d == 0,
                                     stop=d == ND - 1)
                return ps

            def emit_vtp(vt_sb, l, sub):
                # vT chunk -> natural layout via PE transpose
                for s4 in range(0, SUB // 128, 2):
                    tp = scr.tile([128, 256], DT, tag="scr", name="tp")
                    nc.tensor.transpose(
                        tp[:, 0:128],
                        vt_sb[:, s4 * 128:(s4 + 1) * 128], ident[:])
                    nc.tensor.transpose(
                        tp[:, 128:256],
                        vt_sb[:, (s4 + 1) * 128:(s4 + 2) * 128], ident[:])
                    col = sub * SUB + s4 * 128
                    if s4 == 0:
                        nc.scalar.copy(vN[l][:, col:col + 256], tp[:, 0:256])
                    else:
                        nc.vector.tensor_copy(vN[l][:, col:col + 256],
                                              tp[:, 0:256])

            xt_cur = xt0
            for sub in range(NSUB):
                xt = xt_cur
                if sub + 1 < NSUB:
                    xt_cur = stage_xt(sub + 1)
                if sub == 1:
                    # wo is first needed at the out-projection (~attention
                    # start); load it in the shadow of projection compute
                    for lh in range(0, HL, 2):
                        nc.sync.dma_start(out=wo_sb[:, lh:lh + 2, :],
                                          in_=wop[:, lh:lh + 2, :])
                sl = slice(sub * SUB, (sub + 1) * SUB)
                for l in range(HL):
                    q_ps = chain(wq_sb, l, xt)
                    nc.vector.tensor_copy(qT[l][:, sl], q_ps[:])
                for l in range(HL):
                    k_ps = chain(wk_sb, l, xt)
                    nc.vector.tensor_copy(kT[l][:, sl], k_ps[:])
                    # rope rotation matmuls ride between k chains
                    rot = scr.tile([128, SUB], F32, tag="scr")
                    nc.tensor.matmul(rot[0:64, :], p64[64:128, :],
                                     qT[l][64:128, sl], start=True, stop=True)
                    tmp = small.tile([128, SUB], F32, tag="ropetmp")
                    nc.vector.tensor_tensor(qT[l][64:128, sl],
                                            qT[l][64:128, sl],
                                            cosT[64:128, sl], op=ALU.mult)
                    nc.vector.tensor_tensor(tmp[64:128, :], rot[0:64, :],
                                            sinT[64:128, sl], op=ALU.mult)
                    nc.vector.tensor_tensor(qT[l][64:128, sl],
                                            qT[l][64:128, sl],
                                            tmp[64:128, :], op=ALU.add)
                    if l == 0:
                        rotk = scr.tile([128, SUB], F32, tag="scr")
                        nc.tensor.matmul(rotk[0:64, :], p64[64:128, :],
                                         kT[0][64:128, sl],
                                         start=True, stop=True)
                        tmpk = small.tile([128, SUB], F32, tag="ropetmp")
                        nc.vector.tensor_tensor(kT[0][64:128, sl],
                                                kT[0][64:128, sl],
                                                cosT[64:128, sl], op=ALU.mult)
                        nc.vector.tensor_tensor(tmpk[64:128, :], rotk[0:64, :],
                                                sinT[64:128, sl], op=ALU.mult)
                        nc.vector.tensor_tensor(kT[0][64:128, sl],
                                                kT[0][64:128, sl],
                                                tmpk[64:128, :], op=ALU.add)
                    else:
                        # shared rope key: reuse head 0's rotated rows
                        nc.vector.tensor_copy(kT[l][64:128, sl],
                                              kT[0][64:128, sl])
                vts = []
                for l in range(HL):
                    v_ps = chain(wv_sb, l, xt)
                    vt_sb = small.tile([128, SUB], DT, tag="vtsb", bufs=3)
                    nc.scalar.copy(vt_sb[:], v_ps[:])
                    vts.append(vt_sb)
                    if l > 0:
                        emit_vtp(vts[l - 1], l - 1, sub)
                emit_vtp(vts[HL - 1], HL - 1, sub)

            # -------- attention (quads of 4 q-blocks) + out-projection -----
            LOOK = 3
            ctx_hist = {}

            def outproj_qh(q, qh):
                row0 = (4 * q + qh) * 128
                for n in range(4):
                    op_ps = psA.tile([128, 512], F32, tag="mm")
                    for l in range(HL):
                        nc.tensor.matmul(
                            op_ps[:],
                            ctx_hist[q][l][:, qh * 128:(qh + 1) * 128],
                            wo_sb[:, l, n * 512:(n + 1) * 512],
                            start=l == 0, stop=l == HL - 1)
                    ob = stream.tile([128, 512], ODT, tag="ob")
                    if n % 2 == 0:
                        nc.scalar.copy(ob[:], op_ps[:])
                    else:
                        nc.vector.tensor_copy(ob[:], op_ps[:])
                    nc.sync.dma_start(
                        out=outd[row0:row0 + 128, n * 512:(n + 1) * 512],
                        in_=ob[:])

            # finalize (softmax denominator + normalized ctx copy) for head
            # (q, l) is DEFERRED into the next head's section, split in two:
            # part A (DVE merge + PE rowsum matmul) lands early where its
            # inputs are long ready; part B (ACT Ln/Exp + DVE mult) lands
            # after the section's scores so it sits in the ACT idle slot
            # instead of delaying the next head's Exp stream.
            pend_a = [None]
            pend_b = [None]
            pend_tail = [None]      # deferred AV tail of the previous head

            def fin_a():
                st = pend_a[0]
                if st is None:
                    return
                pend_a[0] = None
                fq, fl, acc, racg, racv = st
                rbf = small.tile([128, 512], DT, tag="rbf")
                nc.vector.tensor_tensor(rbf[:], racg[:], racv[:], op=ALU.add)
                rs = accp.tile([128, 512], F32, tag="rs", bufs=1, name="rs")
                nc.tensor.matmul(rs[:], ones[:], rbf[:], start=True,
                                 stop=True)
                # drain the rowsum bank right away so its single PSUM buf
                # never serializes the next head's rowsum matmul
                rs_sb = small.tile([128, 512], F32, tag="rssb")
                nc.vector.tensor_copy(rs_sb[:], rs[:])
                pend_b[0] = (fq, fl, acc, rs_sb)

            def fin_b():
                st = pend_b[0]
                if st is None:
                    return
                pend_b[0] = None
                fq, fl, acc, rs_sb = st
                lnr = small.tile([128, 512], F32, tag="lnr")
                nc.scalar.activation(lnr[:], rs_sb[:], AF.Ln)
                rc = small.tile([128, 512], F32, tag="rc")
                nc.scalar.activation(rc[:], lnr[:], AF.Exp, scale=-1.0)
                ct = ctxp.tile([128, 512], DT, tag=f"ct{fl}")
                nc.vector.tensor_tensor(ct[:], acc[:], rc[:], op=ALU.mult)
                ctx_hist[fq].append(ct)

            for q in range(NQUAD):
                q0 = 4 * q                      # first q-block of the quad
                nbk = 4 * q + 4                 # k blocks 0..nbk-1
                ctx_hist[q] = []
                for l in range(HL):
                    acc = accp.tile([128, 512], F32, tag="acc")
                    # exp-sum side chains, split even/odd blocks across the
                    # GpSimd and Vector engines (GpSimd alone can't keep pace)
                    racg = small.tile([128, 512], F32, tag="racg")
                    racv = small.tile([128, 512], F32, tag="racv",
                                      name="racv")
                    at_tiles = [None] * nbk

                    def emit_score(kb):
                        i0 = max(kb - q0, 0)    # first valid q-block index
                        W = 512 - 128 * i0
                        c0 = 512 - W
                        sT = scr.tile([128, 512], F32, tag="scr")
                        nc.tensor.matmul(
                            sT[:, 0:W],
                            kT[l][:, kb * 128:(kb + 1) * 128],
                            qT[l][:, q0 * 128 + c0: (q0 + 4) * 128],
                            start=True, stop=True)
                        at = atp.tile([128, 512], DT, tag="at")
                        nc.scalar.activation(at[:, 0:W], sT[:, 0:W], AF.Exp)
                        if kb >= q0:
                            # diagonal q-block sits at tile cols 0:128
                            nc.vector.tensor_tensor(
                                at[:, 0:128], at[:, 0:128], tri[:],
                                op=ALU.mult)
                        eng, r = (nc.gpsimd, racg) if kb % 2 == 0 \
                            else (nc.vector, racv)
                        if kb < 2:
                            if c0 > 0:
                                eng.memset(r[:, 0:c0], 0.0)
                            eng.tensor_copy(r[:, c0:512], at[:, 0:W])
                        else:
                            eng.tensor_tensor(r[:, c0:512], r[:, c0:512],
                                              at[:, 0:W], op=ALU.add)
                        at_tiles[kb] = at

                    def emit_av(kb, acc=acc, at_tiles=at_tiles, l=l, q0=q0):
                        i0 = max(kb - q0, 0)
                        W = 512 - 128 * i0
                        c0 = 512 - W
                        at = at_tiles[kb]
                        vblk = vN[l][:, kb * 128:(kb + 1) * 128]
                        if kb < q0:
                            nc.tensor.matmul(acc[:, 0:512], vblk, at[:, 0:512],
                                             start=kb == 0, stop=False)
                        else:
                            # diag block: cols 0:128 of the tile finish
                            # q-block i0; the rest continue accumulating
                            nc.tensor.matmul(
                                acc[:, c0:c0 + 128], vblk, at[:, 0:128],
                                start=kb == 0, stop=True)
                            if W > 128:
                                nc.tensor.matmul(
                                    acc[:, c0 + 128:512], vblk, at[:, 128:W],
                                    start=kb == 0, stop=i0 == 3)

                    tail = pend_tail[0]
                    pend_tail[0] = None
                    for kb in range(nbk):
                        emit_score(kb)
                        # the previous head's AV tail rides in the first
                        # score slots, where its exps are long done
                        if tail and kb < len(tail):
                            tail[kb]()
                        if kb == 1:
                            fin_a()
                        if kb >= LOOK:
                            emit_av(kb - LOOK)
                    pend_tail[0] = [functools.partial(emit_av, kb)
                                    for kb in range(nbk - LOOK, nbk)]

                    fin_b()
                    pend_a[0] = (q, l, acc, racg, racv)
                    # interleave one q-block of the previous quad's
                    # out-projection per head: PE burst with no ACT deps,
                    # so the Exp pipeline gets a catch-up window
                    if q > 0:
                        outproj_qh(q - 1, l)
            for f in pend_tail[0] or []:
                f()
            fin_a()
            fin_b()
            for qh in range(4):
                outproj_qh(NQUAD - 1, qh)

    return nc


# ---------------------------------------------------------------------------
# Host side
# ---------------------------------------------------------------------------
def _rope_tables():
    inv_freq = 1.0 / (10000.0 ** (np.arange(0, RD, 2, dtype=np.float32) / RD))
    t = np.arange(S, dtype=np.float32)
    freqs = np.outer(t, inv_freq).astype(np.float32)
    emb = np.concatenate([freqs, freqs], axis=-1)
    cos = np.cos(emb).astype(np.float32)    # [S, RD]
    sin = np.sin(emb).astype(np.float32)
    return np.ascontiguousarray(cos.T), np.ascontiguousarray(sin.T)


def _host_prep(x, W_kv_down, W_q_down, W_kc, W_v, W_qc, W_kr, W_qr, W_o, b_o):
    f = np.float32
    Wqc_f = (W_q_down @ W_qc).astype(f)       # [D, CD*H]
    Wqr_f = (W_q_down @ W_qr).astype(f)       # [D, RD*H]
    Wkc_f = (W_kv_down @ W_kc).astype(f)      # [D, CD*H]
    Wv_f = (W_kv_down @ W_v).astype(f)        # [D, HD*H]
    scale = f(1.0 / np.sqrt(np.float32(HD)))

    cosT, sinT = _rope_tables()

    ident = np.eye(128, dtype=f)
    p64 = np.zeros((RD, RD), f)
    for m in range(RD):
        if m < 32:
            p64[m + 32, m] = -1.0
        else:
            p64[m - 32, m] = 1.0
    tri01 = (np.arange(128)[:, None] <= np.arange(128)[None, :]).astype(f)
    ones128 = np.ones((128, 128), f)

    # packed per-group weights: [128, ND, HL*128]
    wq_g, wk_g, wv_g, wo_g = [], [], [], []
    for g in range(NG):
        wq_c = np.empty((D, HL * HD), f)
        wk_c = np.empty((D, HL * HD), f)
        wv_c = np.empty((D, HL * HD), f)
        wo_c = np.empty((HL * HD, D), f)
        for l in range(HL):
            h = HL * g + l
            wq_c[:, l * 128:l * 128 + 64] = \
                Wqc_f[:, h * 64:(h + 1) * 64] * scale
            wq_c[:, l * 128 + 64:(l + 1) * 128] = \
                Wqr_f[:, h * 64:(h + 1) * 64] * scale
            wk_c[:, l * 128:l * 128 + 64] = Wkc_f[:, h * 64:(h + 1) * 64]
            wk_c[:, l * 128 + 64:(l + 1) * 128] = W_kr
            wv_c[:, l * 128:(l + 1) * 128] = Wv_f[:, h * 128:(h + 1) * 128]
            wo_c[l * 128:(l + 1) * 128, :] = W_o[h * 128:(h + 1) * 128, :]
        # [D, C] -> [128, ND, C]
        wq_g.append(np.ascontiguousarray(
            wq_c.reshape(ND, 128, HL * HD).transpose(1, 0, 2)))
        wk_g.append(np.ascontiguousarray(
            wk_c.reshape(ND, 128, HL * HD).transpose(1, 0, 2)))
        wv_g.append(np.ascontiguousarray(
            wv_c.reshape(ND, 128, HL * HD).transpose(1, 0, 2)))
        # [HL*128, D] -> [128, HL, D]
        wo_g.append(np.ascontiguousarray(
            wo_c.reshape(HL, 128, D).transpose(1, 0, 2)))

    # packed x per batch: [128, ND, S];  xtp[p, d, t] = x[b, t, d*128+p]
    xtp_b = []
    for b in range(NB):
        xb = x[b]                              # [S, D]
        xtp_b.append(np.ascontiguousarray(
            xb.T.reshape(ND, 128, S).transpose(1, 0, 2)))

    in_maps = []
    for c in range(NC):
        b, g = c // NG, c % NG
        in_maps.append({
            "xtp": xtp_b[b], "wqp": wq_g[g], "wkp": wk_g[g],
            "wvp": wv_g[g], "wop": wo_g[g],
            "cosd": cosT, "sind": sinT,
            "identd": ident, "p64d": p64, "trid": tri01, "onesd": ones128,
        })
    if _DT_NAME == "bf16":
        import ml_dtypes
        bf = ml_dtypes.bfloat16
        in_maps = [{k: v.astype(bf) for k, v in m.items()} for m in in_maps]
    return in_maps


def kernel(**inputs):
    inputs = {k: np.asarray(v, np.float32) for k, v in inputs.items()}
    if "nc" not in _CACHE:
        prog = _build_program()
        _split_multi_waits(prog)
        _CACHE["nc"] = prog
    prog = _CACHE["nc"]
    in_maps = _host_prep(**inputs)
    res = None
    for attempt in range(3):
        try:
            res = run_bass_kernel_spmd(prog, in_maps, core_ids=list(range(NC)))
            break
        except Exception:
            if attempt == 2:
                raise
            import time
            time.sleep(5.0)
    out = np.zeros((B, S, D), np.float32)
    for c, r in enumerate(res.results):
        out[c // NG] += np.asarray(r["out"], np.float32)
    out += inputs["b_o"][None, None, :]
    return out


# revision 48
# speedup vs baseline: 1.0095x; 1.0095x over previous
"""MultiHeadLatentAttention TRN2 kernel — 8-core batch(2) x head-group(4) parallel.

v2 strategy (rewrite of the 534us head-sharded baseline):
  - Sharding: core c handles batch c//4 and heads 4*(c%4)..4*(c%4)+3.
    Per-core x input halves (8.4MB) and the out-projection partial halves
    ([2048, 2048] instead of [4096, 2048]); host sums 4 partials per batch.
  - Host fuses latent down-projections into the up-projections (weight
    absorption) exactly like v1; 1/sqrt(HD) folded into q weights. All
    device operands pre-packed into their SBUF layouts so every DMA moves
    contiguous >=2KB rows; DMA issue order is arranged so the first
    projection matmul is gated on ~1.5MB, not the full weight set.
  - Projections: per 512-token sub-chunk, 12 accumulation chains
    (4 heads x q/k/v) at N=512. RoPE rotation via PE permutation matmul +
    DVE mul/add; the rotated shared rope-key is computed once (head 0) and
    copied to heads 1-3 instead of re-roped.
  - Attention: scores are computed TRANSPOSED (scoresT[k,q] = kT_blk^T @ qT)
    in quads of 4 q-blocks (N up to 512), exp'd on ACT straight into bf16
    SBUF, causal-masked multiplicatively (0/1 tri), and fed directly to the
    AV matmul (ctxT[hd,q] += vN_blk^T @ expT) — no attention transposes, no
    PSUM round-trips. Softmax denominators: either GpSimd
    partition_all_reduce over a DVE-accumulated exp-sum (MLA_RS=gp, default)
    or M=1 ones-matmuls on PE (MLA_RS=pe). Normalization is folded into the
    ctx PSUM->SBUF copy (tensor_tensor mult with the broadcast reciprocal).
  - Out-projection per quad, delayed by one quad so its PE burst overlaps
    the next quad's softmax latency; partials stream out as bf16
    (MLA_OUT=bf16 default; f32 fallback).
"""
import functools
import os
import numpy as np

import concourse.bass as bass
import concourse.mybir as mybir
from concourse.tile import TileContext
from concourse.bass_utils import run_bass_kernel_spmd

F32 = mybir.dt.float32
AF = mybir.ActivationFunctionType
ALU = mybir.AluOpType

NC = 8           # cores
NB = 2           # batch shards
NG = 4           # head-group shards
HL = 4           # heads per core
B, S, D = 2, 2048, 2048
H, HD, RD, CD = 16, 128, 64, 64
ND = D // 128    # 16 contraction chunks
SUB = 512        # projection sub-chunk (tokens, = PSUM N = xt stage)
NSUB = S // SUB  # 4
QB = 128
NQB = S // QB    # 16
NQUAD = NQB // 4  # 4 quads of 4 q-blocks

_DT_NAME = os.environ.get("MLA_DT", "bf16")     # bf16 | f32
_OUT_NAME = os.environ.get("MLA_OUT", "bf16")   # bf16 | f32

_CACHE = {}


# ---------------------------------------------------------------------------
# Tile-on-this-walrus compat: max ONE sync wait per instruction. Extra waits
# are hoisted onto wait-only EventSemaphore instructions inserted just before
# the over-subscribed instruction on the same engine (program order makes
# this equivalent).
# ---------------------------------------------------------------------------
def _split_multi_waits(nc, max_waits=1):
    n = 0
    for f in nc.m.functions:
        for bb in f.blocks:
            new_insts = []
            for ins in bb.instructions:
                si = ins.sync_info
                waits = list(si.on_wait) if si is not None else []
                if len(waits) > max_waits:
                    extra, keep = waits[:-max_waits], waits[-max_waits:]
                    for j, w in enumerate(extra):
                        ev = mybir.InstEventSemaphore(
                            name=f"{ins.name}_xw{j}",
                            engine=ins.engine,
                            ins=[],
                            outs=[],
                            sync_info=mybir.SyncInfo(on_wait=[w], on_update=[]),
                        )
                        new_insts.append(ev)
                        n += 1
                    ins.sync_info = mybir.SyncInfo(
                        on_wait=keep, on_update=list(si.on_update)
                    )
                new_insts.append(ins)
            bb.instructions[:] = new_insts
    return n


def _stream_dt():
    return mybir.dt.bfloat16 if _DT_NAME == "bf16" else mybir.dt.float32


def _out_dt():
    return mybir.dt.bfloat16 if _OUT_NAME == "bf16" else mybir.dt.float32


# ---------------------------------------------------------------------------
# Device program (SPMD: identical on all 8 cores, inputs differ per core)
# ---------------------------------------------------------------------------
def _build_program():
    DT = _stream_dt()
    ODT = _out_dt()
    nc = bass.Bass()

    xtp = nc.dram_tensor("xtp", [128, ND, S], DT, kind="ExternalInput")
    wqp = nc.dram_tensor("wqp", [128, ND, HL * HD], DT, kind="ExternalInput")
    wkp = nc.dram_tensor("wkp", [128, ND, HL * HD], DT, kind="ExternalInput")
    wvp = nc.dram_tensor("wvp", [128, ND, HL * HD], DT, kind="ExternalInput")
    wop = nc.dram_tensor("wop", [128, HL, D], DT, kind="ExternalInput")
    cosd = nc.dram_tensor("cosd", [RD, S], DT, kind="ExternalInput")
    sind = nc.dram_tensor("sind", [RD, S], DT, kind="ExternalInput")
    identd = nc.dram_tensor("identd", [128, 128], DT, kind="ExternalInput")
    p64d = nc.dram_tensor("p64d", [RD, RD], DT, kind="ExternalInput")
    trid = nc.dram_tensor("trid", [128, 128], DT, kind="ExternalInput")
    onesd = nc.dram_tensor("onesd", [128, 128], DT, kind="ExternalInput")
    outd = nc.dram_tensor("out", [S, D], ODT, kind="ExternalOutput")

    with TileContext(nc) as tc:
        with tc.tile_pool(name="stat", bufs=1) as stat, \
             tc.tile_pool(name="seq", bufs=1) as seq, \
             tc.tile_pool(name="xtc", bufs=2) as xtc, \
             tc.tile_pool(name="atp", bufs=9) as atp, \
             tc.tile_pool(name="ctxp", bufs=2) as ctxp, \
             tc.tile_pool(name="stream", bufs=4) as stream, \
             tc.tile_pool(name="small", bufs=2) as small, \
             tc.tile_pool(name="psA", bufs=2, space="PSUM") as psA, \
             tc.tile_pool(name="scr", bufs=3, space="PSUM") as scr, \
             tc.tile_pool(name="accp", bufs=2, space="PSUM") as accp:

            # ---- staged constants/weights; DMA order = priority order ----
            # (first projection chain is gated only on xt[d0:4] + wq[d0:4])
            def stage_xt(sub):
                xt = xtc.tile([128, ND, SUB], DT, tag="xt", name=f"xt{sub}")
                csl = slice(sub * SUB, (sub + 1) * SUB)
                for dpart in range(0, ND, 4):
                    dsl = slice(dpart, dpart + 4)
                    nc.sync.dma_start(out=xt[:, dsl, :], in_=xtp[:, dsl, csl])
                return xt

            wq_sb = stat.tile([128, ND, HL * HD], DT, tag="wq")
            wk_sb = stat.tile([128, ND, HL * HD], DT, tag="wk")
            wv_sb = stat.tile([128, ND, HL * HD], DT, tag="wv")
            xt0 = xtc.tile([128, ND, SUB], DT, tag="xt", name="xt0")
            for dpart in range(0, ND, 4):
                dsl = slice(dpart, dpart + 4)
                nc.sync.dma_start(out=xt0[:, dsl, :], in_=xtp[:, dsl, 0:SUB])
                nc.sync.dma_start(out=wq_sb[:, dsl, :], in_=wqp[:, dsl, :])
                nc.sync.dma_start(out=wk_sb[:, dsl, :], in_=wkp[:, dsl, :])
                nc.sync.dma_start(out=wv_sb[:, dsl, :], in_=wvp[:, dsl, :])
            cosT = stat.tile([128, S], DT, tag="cos")
            sinT = stat.tile([128, S], DT, tag="sin")
            nc.sync.dma_start(out=cosT[64:128, :], in_=cosd[:])
            nc.sync.dma_start(out=sinT[64:128, :], in_=sind[:])
            ident = stat.tile([128, 128], DT, tag="ident")
            nc.sync.dma_start(out=ident[:], in_=identd[:])
            p64 = stat.tile([128, RD], DT, tag="p64")
            nc.sync.dma_start(out=p64[64:128, :], in_=p64d[:])
            tri = stat.tile([128, 128], DT, tag="tri")
            nc.sync.dma_start(out=tri[:], in_=trid[:])
            ones = stat.tile([128, 128], DT, tag="ones")
            nc.sync.dma_start(out=ones[:], in_=onesd[:])
            wo_sb = stat.tile([128, HL, D], DT, tag="wo")

            qT = [seq.tile([128, S], DT, tag=f"qT{l}", name=f"qT{l}")
                  for l in range(HL)]
            kT = [seq.tile([128, S], DT, tag=f"kT{l}", name=f"kT{l}")
                  for l in range(HL)]
            vN = [seq.tile([128, S], DT, tag=f"vN{l}", name=f"vN{l}")
                  for l in range(HL)]

            # ---------------- projections ----------------
            # chain order per sub: q(0..3), k(0..3), v(0..3) — the first 4
            # chains are gated only on wq+xt (4MB); wk/wv stream in their
            # shadow. RoPE rotations ride along the k chains; v transposes
            # are deferred one chain so the vt drain never stalls PE.
            def chain(w_sb, l, xt):
                ps = psA.tile([128, SUB], F32, tag="mm", name="ps")
                for d in range(ND):
                    nc.tensor.matmul(ps[:], w_sb[:, d, l * 128:(l + 1) * 128],
                                     xt[:, d, :], start=d == 0,
                                     stop=d == ND - 1)
                return ps

            def emit_vtp(vt_sb, l, sub):
                # vT chunk -> natural layout via PE transpose
                for s4 in range(0, SUB // 128, 2):
                    tp = scr.tile([128, 256], DT, tag="scr", name="tp")
                    nc.tensor.transpose(
                        tp[:, 0:128],
                        vt_sb[:, s4 * 128:(s4 + 1) * 128], ident[:])
                    nc.tensor.transpose(
                        tp[:, 128:256],
                        vt_sb[:, (s4 + 1) * 128:(s4 + 2) * 128], ident[:])
                    col = sub * SUB + s4 * 128
                    if s4 == 0:
                        nc.scalar.copy(vN[l][:, col:col + 256], tp[:, 0:256])
                    else:
                        nc.vector.tensor_copy(vN[l][:, col:col + 256],
                                              tp[:, 0:256])

            def emit_proj_sub(sub, xt):
                sl = slice(sub * SUB, (sub + 1) * SUB)
                for l in range(HL):
                    q_ps = chain(wq_sb, l, xt)
                    nc.vector.tensor_copy(qT[l][:, sl], q_ps[:])
                for l in range(HL):
                    k_ps = chain(wk_sb, l, xt)
                    nc.vector.tensor_copy(kT[l][:, sl], k_ps[:])
                    # rope rotation matmuls ride between k chains
                    rot = scr.tile([128, SUB], F32, tag="scr")
                    nc.tensor.matmul(rot[0:64, :], p64[64:128, :],
                                     qT[l][64:128, sl], start=True, stop=True)
                    tmp = small.tile([128, SUB], F32, tag="ropetmp")
                    nc.vector.tensor_tensor(qT[l][64:128, sl],
                                            qT[l][64:128, sl],
                                            cosT[64:128, sl], op=ALU.mult)
                    nc.vector.tensor_tensor(tmp[64:128, :], rot[0:64, :],
                                            sinT[64:128, sl], op=ALU.mult)
                    nc.vector.tensor_tensor(qT[l][64:128, sl],
                                            qT[l][64:128, sl],
                                            tmp[64:128, :], op=ALU.add)
                    if l == 0:
                        rotk = scr.tile([128, SUB], F32, tag="scr")
                        nc.tensor.matmul(rotk[0:64, :], p64[64:128, :],
                                         kT[0][64:128, sl],
                                         start=True, stop=True)
                        tmpk = small.tile([128, SUB], F32, tag="ropetmp")
                        nc.vector.tensor_tensor(kT[0][64:128, sl],
                                                kT[0][64:128, sl],
                                                cosT[64:128, sl], op=ALU.mult)
                        nc.vector.tensor_tensor(tmpk[64:128, :], rotk[0:64, :],
                                                sinT[64:128, sl], op=ALU.mult)
                        nc.vector.tensor_tensor(kT[0][64:128, sl],
                                                kT[0][64:128, sl],
                                                tmpk[64:128, :], op=ALU.add)
                    else:
                        # shared rope key: reuse head 0's rotated rows
                        nc.vector.tensor_copy(kT[l][64:128, sl],
                                              kT[0][64:128, sl])
                vts = []
                for l in range(HL):
                    v_ps = chain(wv_sb, l, xt)
                    vt_sb = small.tile([128, SUB], DT, tag="vtsb", bufs=3)
                    nc.scalar.copy(vt_sb[:], v_ps[:])
                    vts.append(vt_sb)
                    if l > 0:
                        emit_vtp(vts[l - 1], l - 1, sub)
                emit_vtp(vts[HL - 1], HL - 1, sub)

            # -------- attention (quads of 4 q-blocks) + out-projection -----
            LOOK = 3
            ctx_hist = {}

            def outproj_qh(q, qh):
                row0 = (4 * q + qh) * 128
                for n in range(4):
                    op_ps = psA.tile([128, 512], F32, tag="mm")
                    for l in range(HL):
                        nc.tensor.matmul(
                            op_ps[:],
                            ctx_hist[q][l][:, qh * 128:(qh + 1) * 128],
                            wo_sb[:, l, n * 512:(n + 1) * 512],
                            start=l == 0, stop=l == HL - 1)
                    ob = stream.tile([128, 512], ODT, tag="ob")
                    if n % 2 == 0:
                        nc.scalar.copy(ob[:], op_ps[:])
                    else:
                        nc.vector.tensor_copy(ob[:], op_ps[:])
                    nc.sync.dma_start(
                        out=outd[row0:row0 + 128, n * 512:(n + 1) * 512],
                        in_=ob[:])

            # finalize (softmax denominator + normalized ctx copy) for head
            # (q, l) is DEFERRED into the next head's section, split in two:
            # part A (DVE merge + PE rowsum matmul) lands early where its
            # inputs are long ready; part B (ACT Ln/Exp + DVE mult) lands
            # after the section's scores so it sits in the ACT idle slot
            # instead of delaying the next head's Exp stream.
            pend_a = [None]
            pend_b = [None]
            pend_tail = [None]      # deferred AV tail of the previous head

            def fin_a():
                st = pend_a[0]
                if st is None:
                    return
                pend_a[0] = None
                fq, fl, acc, racg, racv = st
                rbf = small.tile([128, 512], DT, tag="rbf")
                nc.vector.tensor_tensor(rbf[:], racg[:], racv[:], op=ALU.add)
                rs = accp.tile([128, 512], F32, tag="rs", bufs=1, name="rs")
                nc.tensor.matmul(rs[:], ones[:], rbf[:], start=True,
                                 stop=True)
                # drain the rowsum bank right away so its single PSUM buf
                # never serializes the next head's rowsum matmul
                rs_sb = small.tile([128, 512], F32, tag="rssb")
                nc.vector.tensor_copy(rs_sb[:], rs[:])
                pend_b[0] = (fq, fl, acc, rs_sb)

            def fin_b():
                st = pend_b[0]
                if st is None:
                    return
                pend_b[0] = None
                fq, fl, acc, rs_sb = st
                lnr = small.tile([128, 512], F32, tag="lnr")
                nc.scalar.activation(lnr[:], rs_sb[:], AF.Ln)
                rc = small.tile([128, 512], F32, tag="rc")
                nc.scalar.activation(rc[:], lnr[:], AF.Exp, scale=-1.0)
                ct = ctxp.tile([128, 512], DT, tag=f"ct{fl}")
                nc.vector.tensor_tensor(ct[:], acc[:], rc[:], op=ALU.mult)
                ctx_hist[fq].append(ct)

            def emit_attn_quad(q):
                q0 = 4 * q                      # first q-block of the quad
                nbk = 4 * q + 4                 # k blocks 0..nbk-1
                ctx_hist[q] = []
                for l in range(HL):
                    acc = accp.tile([128, 512], F32, tag="acc")
                    # exp-sum side chains, split even/odd blocks across the
                    # GpSimd and Vector engines (GpSimd alone can't keep pace)
                    racg = small.tile([128, 512], F32, tag="racg")
                    racv = small.tile([128, 512], F32, tag="racv",
                                      name="racv")
                    at_tiles = [None] * nbk

                    def emit_score(kb):
                        i0 = max(kb - q0, 0)    # first valid q-block index
                        W = 512 - 128 * i0
                        c0 = 512 - W
                        sT = scr.tile([128, 512], F32, tag="scr")
                        nc.tensor.matmul(
                            sT[:, 0:W],
                            kT[l][:, kb * 128:(kb + 1) * 128],
                            qT[l][:, q0 * 128 + c0: (q0 + 4) * 128],
                            start=True, stop=True)
                        at = atp.tile([128, 512], DT, tag="at")
                        nc.scalar.activation(at[:, 0:W], sT[:, 0:W], AF.Exp)
                        if kb >= q0:
                            # diagonal q-block sits at tile cols 0:128
                            nc.vector.tensor_tensor(
                                at[:, 0:128], at[:, 0:128], tri[:],
                                op=ALU.mult)
                        eng, r = (nc.gpsimd, racg) if kb % 2 == 0 \
                            else (nc.vector, racv)
                        if kb < 2:
                            if c0 > 0:
                                eng.memset(r[:, 0:c0], 0.0)
                            eng.tensor_copy(r[:, c0:512], at[:, 0:W])
                        else:
                            eng.tensor_tensor(r[:, c0:512], r[:, c0:512],
                                              at[:, 0:W], op=ALU.add)
                        at_tiles[kb] = at

                    def emit_av(kb, acc=acc, at_tiles=at_tiles, l=l, q0=q0):
                        i0 = max(kb - q0, 0)
                        W = 512 - 128 * i0
                        c0 = 512 - W
                        at = at_tiles[kb]
                        vblk = vN[l][:, kb * 128:(kb + 1) * 128]
                        if kb < q0:
                            nc.tensor.matmul(acc[:, 0:512], vblk, at[:, 0:512],
                                             start=kb == 0, stop=False)
                        else:
                            # diag block: cols 0:128 of the tile finish
                            # q-block i0; the rest continue accumulating
                            nc.tensor.matmul(
                                acc[:, c0:c0 + 128], vblk, at[:, 0:128],
                                start=kb == 0, stop=True)
                            if W > 128:
                                nc.tensor.matmul(
                                    acc[:, c0 + 128:512], vblk, at[:, 128:W],
                                    start=kb == 0, stop=i0 == 3)

                    tail = pend_tail[0]
                    pend_tail[0] = None
                    for kb in range(nbk):
                        emit_score(kb)
                        # the previous head's AV tail rides in the first
                        # score slots, where its exps are long done
                        if tail and kb < len(tail):
                            tail[kb]()
                        if kb == 1:
                            fin_a()
                        if kb >= LOOK:
                            emit_av(kb - LOOK)
                    pend_tail[0] = [functools.partial(emit_av, kb)
                                    for kb in range(nbk - LOOK, nbk)]

                    fin_b()
                    pend_a[0] = (q, l, acc, racg, racv)
                    # interleave one q-block of the previous quad's
                    # out-projection per head: PE burst with no ACT deps,
                    # so the Exp pipeline gets a catch-up window
                    if q > 0:
                        outproj_qh(q - 1, l)

            # ---- driver: projection sub q feeds attention quad q; the
            # interleave mixes the PE-heavy/ACT-light projection stream
            # with the ACT-heavy attention stream so Exp never falls
            # behind, and spreads DMA in/out across the whole kernel ----
            xt_cur = xt0
            for q in range(NQUAD):
                xt = xt_cur
                if q + 1 < NSUB:
                    xt_cur = stage_xt(q + 1)
                if q == 1:
                    # wo is first needed at the out-projection (quad 1);
                    # load it in the shadow of sub-1 projection compute
                    for lh in range(0, HL, 2):
                        nc.sync.dma_start(out=wo_sb[:, lh:lh + 2, :],
                                          in_=wop[:, lh:lh + 2, :])
                emit_proj_sub(q, xt)
                emit_attn_quad(q)
            for f in pend_tail[0] or []:
                f()
            fin_a()
            fin_b()
            for qh in range(4):
                outproj_qh(NQUAD - 1, qh)

    return nc


# ---------------------------------------------------------------------------
# Host side
# ---------------------------------------------------------------------------
def _rope_tables():
    inv_freq = 1.0 / (10000.0 ** (np.arange(0, RD, 2, dtype=np.float32) / RD))
    t = np.arange(S, dtype=np.float32)
    freqs = np.outer(t, inv_freq).astype(np.float32)
    emb = np.concatenate([freqs, freqs], axis=-1)
    cos = np.cos(emb).astype(np.float32)    # [S, RD]
    sin = np.sin(emb).astype(np.float32)
    return np.ascontiguousarray(cos.T), np.ascontiguousarray(sin.T)


def _host_prep(x, W_kv_down, W_q_down, W_kc, W_v, W_qc, W_kr, W_qr, W_o, b_o):
    f = np.float32
    Wqc_f = (W_q_down @ W_qc).astype(f)       # [D, CD*H]
    Wqr_f = (W_q_down @ W_qr).astype(f)       # [D, RD*H]
    Wkc_f = (W_kv_down @ W_kc).astype(f)      # [D, CD*H]
    Wv_f = (W_kv_down @ W_v).astype(f)        # [D, HD*H]
    scale = f(1.0 / np.sqrt(np.float32(HD)))

    cosT, sinT = _rope_tables()

    ident = np.eye(128, dtype=f)
    p64 = np.zeros((RD, RD), f)
    for m in range(RD):
        if m < 32:
            p64[m + 32, m] = -1.0
        else:
            p64[m - 32, m] = 1.0
    tri01 = (np.arange(128)[:, None] <= np.arange(128)[None, :]).astype(f)
    ones128 = np.ones((128, 128), f)

    # packed per-group weights: [128, ND, HL*128]
    wq_g, wk_g, wv_g, wo_g = [], [], [], []
    for g in range(NG):
        wq_c = np.empty((D, HL * HD), f)
        wk_c = np.empty((D, HL * HD), f)
        wv_c = np.empty((D, HL * HD), f)
        wo_c = np.empty((HL * HD, D), f)
        for l in range(HL):
            h = HL * g + l
            wq_c[:, l * 128:l * 128 + 64] = \
                Wqc_f[:, h * 64:(h + 1) * 64] * scale
            wq_c[:, l * 128 + 64:(l + 1) * 128] = \
                Wqr_f[:, h * 64:(h + 1) * 64] * scale
            wk_c[:, l * 128:l * 128 + 64] = Wkc_f[:, h * 64:(h + 1) * 64]
            wk_c[:, l * 128 + 64:(l + 1) * 128] = W_kr
            wv_c[:, l * 128:(l + 1) * 128] = Wv_f[:, h * 128:(h + 1) * 128]
            wo_c[l * 128:(l + 1) * 128, :] = W_o[h * 128:(h + 1) * 128, :]
        # [D, C] -> [128, ND, C]
        wq_g.append(np.ascontiguousarray(
            wq_c.reshape(ND, 128, HL * HD).transpose(1, 0, 2)))
        wk_g.append(np.ascontiguousarray(
            wk_c.reshape(ND, 128, HL * HD).transpose(1, 0, 2)))
        wv_g.append(np.ascontiguousarray(
            wv_c.reshape(ND, 128, HL * HD).transpose(1, 0, 2)))
        # [HL*128, D] -> [128, HL, D]
        wo_g.append(np.ascontiguousarray(
            wo_c.reshape(HL, 128, D).transpose(1, 0, 2)))

    # packed x per batch: [128, ND, S];  xtp[p, d, t] = x[b, t, d*128+p]
    xtp_b = []
    for b in range(NB):
        xb = x[b]                              # [S, D]
        xtp_b.append(np.ascontiguousarray(
            xb.T.reshape(ND, 128, S).transpose(1, 0, 2)))

    in_maps = []
    for c in range(NC):
        b, g = c // NG, c % NG
        in_maps.append({
            "xtp": xtp_b[b], "wqp": wq_g[g], "wkp": wk_g[g],
            "wvp": wv_g[g], "wop": wo_g[g],
            "cosd": cosT, "sind": sinT,
            "identd": ident, "p64d": p64, "trid": tri01, "onesd": ones128,
        })
    if _DT_NAME == "bf16":
        import ml_dtypes
        bf = ml_dtypes.bfloat16
        in_maps = [{k: v.astype(bf) for k, v in m.items()} for m in in_maps]
    return in_maps


def kernel(**inputs):
    inputs = {k: np.asarray(v, np.float32) for k, v in inputs.items()}
    if "nc" not in _CACHE:
        prog = _build_program()
        _split_multi_waits(prog)
        _CACHE["nc"] = prog
    prog = _CACHE["nc"]
    in_maps = _host_prep(**inputs)
    res = None
    for attempt in range(3):
        try:
            res = run_bass_kernel_spmd(prog, in_maps, core_ids=list(range(NC)))
            break
        except Exception:
            if attempt == 2:
                raise
            import time
            time.sleep(5.0)
    out = np.zeros((B, S, D), np.float32)
    for c, r in enumerate(res.results):
        out[c // NG] += np.asarray(r["out"], np.float32)
    out += inputs["b_o"][None, None, :]
    return out
